# revision 7
# baseline (speedup 1.0000x reference)
"""GQA kernel for Trainium2, 8 NeuronCores.

Sharding: core c = b*4 + kv  (b in {0,1} data-parallel over batch,
kv in {0..3} tensor-parallel over the 4 KV head groups; each core owns
4 Q heads + 1 KV head). Each core computes a partial output
x[b] @ Wq[:,kv] -> attention -> @ Wo[kv rows]; host sums the 4 partials
per batch (the row-sharded-Wo all-reduce).

Device layout (per core): everything keyed off transposed activations
xT = x[b].T so all matmuls keep the contraction on the partition dim and
N=256/512 moving operands (fp32r full-rate):
  QT[d,t] = Wq_h^T x^T   (norm+rope applied in-layout)
  S^T[k,q] = K Q^T       (lhsT = K^T tile)
  P = exp(S^T/sqrt(d)) * causal_mask     (no max-subtraction needed:
                                          |S|<=sqrt(d) after RMSNorm)
  L[q] (softmax denom) via ones-matmul (partition reduction + broadcast)
  O^T[d,q] = V^T... (lhsT = V natural [k,d]) accumulated over k blocks
  out_partial[q,:] = O^T.T @ Wo  with 1/L folded in before Wo.
"""

import numpy as np

B, T, D = 2, 2048, 2048
NH, NKV, HD = 16, 4, 128
GQ = NH // NKV            # 4 q heads per kv head
HQ = GQ * HD              # 512 q-dim per core
ROPE_BASE = 500000.0
EPS = 1e-5
SCALE = 1.0 / np.sqrt(HD)
NE = 8                    # phase-1 T eighths
ET = T // NE              # 256
NDC = D // 128            # 16 contraction chunks
NJ = 4                    # phase-2 q chunks of 512
JW = T // NJ              # 512
NKB = T // 128            # 16 k blocks

_cached = {}


def _build_program():
    import concourse.bacc as bacc
    import concourse.mybir as mybir
    from concourse import tile
    from concourse.masks import make_identity

    f32 = mybir.dt.float32
    f32r = mybir.dt.float32r
    AF = mybir.ActivationFunctionType
    from concourse.bass import ts

    nc = bacc.Bacc()

    xt = nc.dram_tensor("xt", [D, T], f32r, kind="ExternalInput")
    wq = nc.dram_tensor("wq", [D, HQ], f32r, kind="ExternalInput")
    wk = nc.dram_tensor("wk", [D, HD], f32r, kind="ExternalInput")
    wv = nc.dram_tensor("wv", [D, HD], f32r, kind="ExternalInput")
    wo = nc.dram_tensor("wo", [HQ, D], f32r, kind="ExternalInput")
    cosd = nc.dram_tensor("cos", [HD, T], f32, kind="ExternalInput")
    sind = nc.dram_tensor("sin", [HD, T], f32, kind="ExternalInput")
    wqcd = nc.dram_tensor("wqc", [HD, 1], f32, kind="ExternalInput")
    wkcd = nc.dram_tensor("wkc", [HD, 1], f32, kind="ExternalInput")
    mskd = nc.dram_tensor("msk", [4, 128, JW], f32r, kind="ExternalInput")
    onesd = nc.dram_tensor("ones", [128, 128], f32r, kind="ExternalInput")
    onesnd = nc.dram_tensor("onesn", [128, 128], f32r, kind="ExternalInput")
    outd = nc.dram_tensor("out", [T, D], f32, kind="ExternalOutput")

    xtr = xt.rearrange("(c p) t -> p c t", p=128)
    wqr = wq.rearrange("(c p) n -> p c n", p=128)
    wkr = wk.rearrange("(c p) n -> p c n", p=128)
    wvr = wv.rearrange("(c p) n -> p c n", p=128)
    wor = wo.rearrange("(c p) n -> p c n", p=128)

    with tile.TileContext(nc) as tc:
        with tc.tile_pool(name="A", bufs=1) as A:
            # persistent across all phases
            QT = A.tile([128, GQ, T], f32r, tag="QT")
            KT = A.tile([128, T], f32r, tag="KT")
            Vn = A.tile([128, NKB, HD], f32r, tag="Vn")
            msk_sb = A.tile([128, 4, JW], f32r, tag="msk")
            ones_sb = A.tile([128, 128], f32r, tag="ones")
            ident = A.tile([128, 128], f32, tag="ident")
            onesn_sb = A.tile([128, 128], f32r, tag="onesn")
            eps_t = A.tile([128, 1], f32, tag="eps")
            nc.vector.memset(eps_t, EPS)
            nc.sync.dma_start(out=ones_sb, in_=onesd[:, :])
            nc.sync.dma_start(out=onesn_sb, in_=onesnd[:, :])
            nc.sync.dma_start(out=msk_sb, in_=mskd.rearrange("r p q -> p r q"))
            make_identity(nc, ident)

            with (
                tc.tile_pool(name="W", bufs=1) as W,
                tc.tile_pool(name="Bp", bufs=1) as Bp,
                tc.tile_pool(name="BX", bufs=2) as BX,
                tc.tile_pool(name="TMP", bufs=3) as TMP,
                tc.tile_pool(name="PS1", bufs=3, space="PSUM") as PS1,
                tc.tile_pool(name="PS1b", bufs=2, space="PSUM") as PS1b,
            ):
                wq_sb = W.tile([128, NDC, HQ], f32r, tag="wq")
                wk_sb = W.tile([128, NDC, HD], f32r, tag="wk")
                wv_sb = W.tile([128, NDC, HD], f32r, tag="wv")
                xt_e0 = BX.tile([128, NDC, ET], f32r, tag="xt")
                for c in range(NDC):
                    nc.sync.dma_start(out=wk_sb[:, c, :], in_=wkr[:, c, :])
                    nc.sync.dma_start(out=xt_e0[:, c, :], in_=xtr[:, c, 0:ET])
                nc.sync.dma_start(out=wv_sb, in_=wvr)
                cos_sb = Bp.tile([128, T], f32, tag="cos")
                sin_sb = Bp.tile([128, T], f32, tag="sin")
                nc.sync.dma_start(out=cos_sb, in_=cosd[:, :])
                nc.sync.dma_start(out=sin_sb, in_=sind[:, :])
                wqc = Bp.tile([128, 1], f32, tag="wqc")
                wkc = Bp.tile([128, 1], f32, tag="wkc")
                nc.sync.dma_start(out=wqc, in_=wqcd[:, :])
                nc.sync.dma_start(out=wkc, in_=wkcd[:, :])
                nc.sync.dma_start(out=wq_sb, in_=wqr)

                def normrope(cpsum, wcol, sl, out_sl):
                    """RMSNorm (over partition dim via ones-matmul broadcast)
                    + norm-weight + RoPE; writes f32r out_sl [128, ET]."""
                    sq = TMP.tile([128, ET], f32r, tag="sq")
                    nc.scalar.activation(sq, cpsum, AF.Square)
                    l2 = PS1b.tile([128, ET], f32, tag="l2")
                    nc.tensor.matmul(l2, onesn_sb, sq, start=True, stop=True)
                    sv = TMP.tile([128, ET], f32, tag="sv")
                    nc.scalar.activation(sv, l2, AF.Sqrt, bias=eps_t)
                    rc = TMP.tile([128, ET], f32, tag="rc")
                    nc.vector.reciprocal(rc, sv)
                    rw = TMP.tile([128, ET], f32, tag="rw")
                    nc.vector.tensor_scalar_mul(rw, rc, wcol)
                    qn = TMP.tile([128, ET], f32, tag="qn")
                    nc.vector.tensor_mul(qn, cpsum, rw)
                    qr = TMP.tile([128, ET], f32, tag="qr")
                    nc.sync.dma_start(out=qr[:64], in_=qn[64:])
                    nc.sync.dma_start(out=qr[64:], in_=qn[:64])
                    t1 = TMP.tile([128, ET], f32, tag="t1")
                    nc.vector.tensor_mul(t1, qn, cos_sb[:, sl])
                    t2 = TMP.tile([128, ET], f32, tag="t2")
                    nc.vector.tensor_mul(t2, qr, sin_sb[:, sl])
                    nc.vector.tensor_sub(out_sl[:64], t1[:64], t2[:64])
                    nc.vector.tensor_add(out_sl[64:], t1[64:], t2[64:])

                for e in range(NE):
                    sl = ts(e, ET)
                    if e == 0:
                        xt_t = xt_e0
                    else:
                        xt_t = BX.tile([128, NDC, ET], f32r, tag="xt")
                        nc.sync.dma_start(out=xt_t, in_=xtr[:, :, sl])
                    # K eighth
                    kp = PS1.tile([128, ET], f32, tag="pp")
                    for c in range(NDC):
                        nc.tensor.matmul(kp, wk_sb[:, c, :], xt_t[:, c, :],
                                         start=(c == 0), stop=(c == NDC - 1))
                    normrope(kp, wkc, sl, KT[:, sl])
                    # V eighth: project transposed then PE-transpose to natural
                    vp = PS1.tile([128, ET], f32, tag="pp")
                    for c in range(NDC):
                        nc.tensor.matmul(vp, wv_sb[:, c, :], xt_t[:, c, :],
                                         start=(c == 0), stop=(c == NDC - 1))
                    vt = TMP.tile([128, ET], f32, tag="vt")
                    nc.scalar.activation(vt, vp, AF.Copy)
                    for i in range(ET // 128):
                        tp = PS1b.tile([128, 128], f32, tag="tp")
                        nc.tensor.transpose(tp, vt[:, ts(i, 128)], ident)
                        nc.scalar.activation(Vn[:, e * (ET // 128) + i, :], tp, AF.Copy)
                    # Q heads
                    for h in range(GQ):
                        qp = PS1.tile([128, ET], f32, tag="pp")
                        for c in range(NDC):
                            nc.tensor.matmul(qp, wq_sb[:, c, ts(h, 128)], xt_t[:, c, :],
                                             start=(c == 0), stop=(c == NDC - 1))
                        normrope(qp, wqc, sl, QT[:, h, sl])

            with (
                tc.tile_pool(name="C", bufs=1) as C,
                tc.tile_pool(name="CP", bufs=4) as CP,
                tc.tile_pool(name="CT", bufs=3) as CT,
                tc.tile_pool(name="CO", bufs=2) as CO,
            ):
                OT = C.tile([128, GQ, T], f32r, tag="OT")
                wo_sb = C.tile([128, GQ, D], f32r, tag="wo")
                for c in range(GQ):
                    nc.sync.dma_start(out=wo_sb[:, c, :], in_=wor[:, c, :])

                with (
                    tc.tile_pool(name="PS2", bufs=2, space="PSUM") as PS2,
                    tc.tile_pool(name="PS2b", bufs=3, space="PSUM") as PS2b,
                ):
                    for h in range(GQ):
                        for J in range(NJ):
                            nkb = 4 * J + 4
                            lp = PS2b.tile([128, JW], f32, tag="l")
                            op = PS2b.tile([128, JW], f32, tag="ot")
                            for kb in range(nkb):
                                sp = PS2.tile([128, JW], f32, tag="s")
                                nc.tensor.matmul(sp, KT[:, ts(kb, 128)],
                                                 QT[:, h, ts(J, JW)],
                                                 start=True, stop=True)
                                P = CP.tile([128, JW], f32r, tag="p")
                                nc.scalar.activation(P, sp, AF.Exp, scale=SCALE)
                                if kb >= 4 * J:
                                    nc.vector.tensor_mul(P, P, msk_sb[:, kb - 4 * J, :])
                                nc.tensor.matmul(lp, ones_sb, P,
                                                 start=(kb == 0), stop=(kb == nkb - 1))
                                nc.tensor.matmul(op, Vn[:, kb, :], P,
                                                 start=(kb == 0), stop=(kb == nkb - 1))
                            rc2 = CT.tile([128, JW], f32, tag="rc2")
                            nc.vector.reciprocal(rc2, lp)
                            nc.vector.tensor_mul(OT[:, h, ts(J, JW)], op, rc2)

                with tc.tile_pool(name="PS3", bufs=4, space="PSUM") as PS3:
                    for qt in range(T // 128):
                        ost = CO.tile([128, D], f32, tag="ost")
                        for c in range(D // JW):
                            oup = PS3.tile([128, JW], f32, tag="op")
                            for hc in range(GQ):
                                nc.tensor.matmul(oup, OT[:, hc, ts(qt, 128)],
                                                 wo_sb[:, hc, ts(c, JW)],
                                                 start=(hc == 0), stop=(hc == GQ - 1))
                            nc.scalar.activation(ost[:, ts(c, JW)], oup, AF.Copy)
                        nc.sync.dma_start(out=outd[qt * 128:(qt + 1) * 128, :], in_=ost)

    nc.finalize()
    return nc


def _host_consts():
    inv = 1.0 / (ROPE_BASE ** (np.arange(0, HD, 2, dtype=np.float64) / HD))
    freqs = np.outer(np.arange(T, dtype=np.float64), inv)
    emb = np.concatenate([freqs, freqs], axis=-1)          # [T, HD]
    cosT = np.ascontiguousarray(np.cos(emb).T.astype(np.float32))  # [HD, T]
    sinT = np.ascontiguousarray(np.sin(emb).T.astype(np.float32))
    msk = np.zeros((4, 128, JW), np.float32)
    for r in range(4):
        k = np.arange(128)[:, None] + 128 * r
        q = np.arange(JW)[None, :]
        msk[r] = (k <= q).astype(np.float32)
    ones = np.ones((128, 128), np.float32)
    return cosT, sinT, msk, ones


def kernel(x, Wq, Wk, Wv, Wo, q_norm_w, k_norm_w):
    from concourse.bass_utils import run_bass_kernel_spmd

    if "nc" not in _cached:
        _cached["nc"] = _build_program()
        _cached["consts"] = _host_consts()
    nc = _cached["nc"]
    cosT, sinT, msk, ones = _cached["consts"]

    x = np.asarray(x, np.float32)
    Wq = np.asarray(Wq, np.float32)
    Wk = np.asarray(Wk, np.float32)
    Wv = np.asarray(Wv, np.float32)
    Wo = np.asarray(Wo, np.float32)
    qw = np.ascontiguousarray(np.asarray(q_norm_w, np.float32).reshape(HD, 1))
    kw = np.ascontiguousarray(np.asarray(k_norm_w, np.float32).reshape(HD, 1))

    xTb = [np.ascontiguousarray(x[b].T) for b in range(B)]
    in_maps = []
    for core in range(8):
        b, kv = divmod(core, NKV)
        in_maps.append({
            "xt": xTb[b],
            "wq": np.ascontiguousarray(Wq[:, kv * HQ:(kv + 1) * HQ]),
            "wk": np.ascontiguousarray(Wk[:, kv * HD:(kv + 1) * HD]),
            "wv": np.ascontiguousarray(Wv[:, kv * HD:(kv + 1) * HD]),
            "wo": np.ascontiguousarray(Wo[kv * HQ:(kv + 1) * HQ, :]),
            "cos": cosT, "sin": sinT, "wqc": qw, "wkc": kw,
            "msk": msk, "ones": ones, "onesn": ones / HD,
        })
    res = run_bass_kernel_spmd(nc, in_maps, list(range(8)))
    out = np.zeros((B, T, D), np.float64)
    for core in range(8):
        b = core // NKV
        out[b] += res.results[core]["out"].astype(np.float64)
    return out.astype(np.float32)


# revision 11
# speedup vs baseline: 27301.4927x; 27301.4927x over previous
"""GQA kernel for Trainium2, 8 NeuronCores.

Sharding: core c = b*4 + kv  (b in {0,1} data-parallel over batch,
kv in {0..3} tensor-parallel over the 4 KV head groups; each core owns
4 Q heads + 1 KV head). Each core computes a partial output
x[b] @ Wq[:,kv] -> attention -> @ Wo[kv rows]; host sums the 4 partials
per batch (the row-sharded-Wo all-reduce).

Device layout (per core): everything keyed off transposed activations
xT = x[b].T so all matmuls keep the contraction on the partition dim and
N=256/512 moving operands (fp32r full-rate):
  QT[d,t] = Wq_h^T x^T   (norm+rope applied in-layout)
  S^T[k,q] = K Q^T       (lhsT = K^T tile)
  P = exp(S^T/sqrt(d)) * causal_mask     (no max-subtraction needed:
                                          |S|<=sqrt(d) after RMSNorm)
  L[q] (softmax denom) via ones-matmul (partition reduction + broadcast)
  O^T[d,q] = V^T... (lhsT = V natural [k,d]) accumulated over k blocks
  out_partial[q,:] = O^T.T @ Wo  with 1/L folded in before Wo.
"""

import numpy as np

B, T, D = 2, 2048, 2048
NH, NKV, HD = 16, 4, 128
GQ = NH // NKV            # 4 q heads per kv head
HQ = GQ * HD              # 512 q-dim per core
ROPE_BASE = 500000.0
EPS = 1e-5
SCALE = 1.0 / np.sqrt(HD)
NE = 8                    # phase-1 T eighths
ET = T // NE              # 256
NDC = D // 128            # 16 contraction chunks
NJ = 4                    # phase-2 q chunks of 512
JW = T // NJ              # 512
NKB = T // 128            # 16 k blocks

_cached = {}


def _build_program():
    import concourse.bacc as bacc
    import concourse.mybir as mybir
    from concourse import tile
    from concourse.masks import make_identity

    f32 = mybir.dt.float32
    f32r = mybir.dt.float32r
    AF = mybir.ActivationFunctionType
    from concourse.bass import ts

    nc = bacc.Bacc()

    xt = nc.dram_tensor("xt", [D, T], f32r, kind="ExternalInput")
    wq = nc.dram_tensor("wq", [D, HQ], f32r, kind="ExternalInput")
    wk = nc.dram_tensor("wk", [D, HD], f32r, kind="ExternalInput")
    wv = nc.dram_tensor("wv", [D, HD], f32r, kind="ExternalInput")
    wo = nc.dram_tensor("wo", [HQ, D], f32r, kind="ExternalInput")
    cosd = nc.dram_tensor("cos", [HD, T], f32, kind="ExternalInput")
    sind = nc.dram_tensor("sin", [HD, T], f32, kind="ExternalInput")
    wqcd = nc.dram_tensor("wqc", [HD, 1], f32, kind="ExternalInput")
    wkcd = nc.dram_tensor("wkc", [HD, 1], f32, kind="ExternalInput")
    wqed = nc.dram_tensor("wqe", [HD, 1], f32, kind="ExternalInput")
    wked = nc.dram_tensor("wke", [HD, 1], f32, kind="ExternalInput")
    mskd = nc.dram_tensor("msk", [4, 128, JW], f32r, kind="ExternalInput")
    onesd = nc.dram_tensor("ones", [128, 128], f32r, kind="ExternalInput")
    onesnd = nc.dram_tensor("onesn", [128, 128], f32r, kind="ExternalInput")
    outd = nc.dram_tensor("out", [T, D], f32, kind="ExternalOutput")

    xtr = xt.rearrange("(c p) t -> p c t", p=128)
    wqr = wq.rearrange("(c p) n -> p c n", p=128)
    wkr = wk.rearrange("(c p) n -> p c n", p=128)
    wvr = wv.rearrange("(c p) n -> p c n", p=128)
    wor = wo.rearrange("(c p) n -> p c n", p=128)

    with tile.TileContext(nc) as tc:
        with tc.tile_pool(name="A", bufs=1) as A:
            # persistent across all phases
            QT = A.tile([128, GQ, T], f32r, tag="QT")
            KT = A.tile([128, T], f32r, tag="KT")
            Vn = A.tile([128, NKB, HD], f32r, tag="Vn")
            msk_sb = A.tile([128, 4, JW], f32r, tag="msk")
            ones_sb = A.tile([128, 128], f32r, tag="ones")
            ident = A.tile([128, 128], f32, tag="ident")
            onesn_sb = A.tile([128, 128], f32r, tag="onesn")
            eps_t = A.tile([128, 1], f32, tag="eps")
            nc.vector.memset(eps_t, EPS)
            nc.sync.dma_start(out=ones_sb, in_=onesd[:, :])
            nc.sync.dma_start(out=onesn_sb, in_=onesnd[:, :])
            nc.sync.dma_start(out=msk_sb, in_=mskd.rearrange("r p q -> p r q"))
            make_identity(nc, ident)

            with (
                tc.tile_pool(name="W", bufs=1) as W,
                tc.tile_pool(name="Bp", bufs=1) as Bp,
                tc.tile_pool(name="BX", bufs=2) as BX,
                tc.tile_pool(name="TMP", bufs=3) as TMP,
                tc.tile_pool(name="PS1", bufs=3, space="PSUM") as PS1,
                tc.tile_pool(name="PS1b", bufs=2, space="PSUM") as PS1b,
            ):
                wq_sb = W.tile([128, NDC, HQ], f32r, tag="wq")
                wk_sb = W.tile([128, NDC, HD], f32r, tag="wk")
                wv_sb = W.tile([128, NDC, HD], f32r, tag="wv")
                xt_e0 = BX.tile([128, NDC, ET], f32r, tag="xt")
                for c in range(NDC):
                    nc.sync.dma_start(out=wk_sb[:, c, :], in_=wkr[:, c, :])
                    nc.sync.dma_start(out=xt_e0[:, c, :], in_=xtr[:, c, 0:ET])
                    nc.sync.dma_start(out=wv_sb[:, c, :], in_=wvr[:, c, :])
                    nc.sync.dma_start(out=wq_sb[:, c, :], in_=wqr[:, c, :])
                cos_sb = Bp.tile([128, T], f32, tag="cos")
                sin_sb = Bp.tile([128, T], f32, tag="sin")
                nc.sync.dma_start(out=cos_sb, in_=cosd[:, :])
                nc.sync.dma_start(out=sin_sb, in_=sind[:, :])
                wqc = Bp.tile([128, 1], f32, tag="wqc")
                wkc = Bp.tile([128, 1], f32, tag="wkc")
                wqe = Bp.tile([128, 1], f32, tag="wqe")
                wke = Bp.tile([128, 1], f32, tag="wke")
                nc.sync.dma_start(out=wqc, in_=wqcd[:, :])
                nc.sync.dma_start(out=wkc, in_=wkcd[:, :])
                nc.sync.dma_start(out=wqe, in_=wqed[:, :])
                nc.sync.dma_start(out=wke, in_=wked[:, :])

                def normrope(cpsum, wcol, wbias, sl, out_sl):
                    """RMSNorm (over partition dim via ones-matmul broadcast)
                    + norm-weight + RoPE; writes f32r out_sl [128, ET]."""
                    sq = TMP.tile([128, ET], f32r, tag="sq")
                    nc.scalar.activation(sq, cpsum, AF.Square)
                    l2 = PS1b.tile([128, ET], f32, tag="l2")
                    nc.tensor.matmul(l2, onesn_sb, sq, start=True, stop=True)
                    sv = TMP.tile([128, ET], f32, tag="sv")
                    nc.scalar.activation(sv, l2, AF.Sqrt, scale=wcol, bias=wbias)
                    rc = TMP.tile([128, ET], f32, tag="rc")
                    nc.vector.reciprocal(rc, sv)
                    qn = TMP.tile([128, ET], f32, tag="qn")
                    nc.vector.tensor_mul(qn, cpsum, rc)
                    qr = TMP.tile([128, ET], f32, tag="qr")
                    nc.sync.dma_start(out=qr[:64], in_=qn[64:])
                    nc.sync.dma_start(out=qr[64:], in_=qn[:64])
                    t1 = TMP.tile([128, ET], f32, tag="t1")
                    nc.vector.tensor_mul(t1, qn, cos_sb[:, sl])
                    t2 = TMP.tile([128, ET], f32, tag="t2")
                    nc.vector.tensor_mul(t2, qr, sin_sb[:, sl])
                    nc.vector.tensor_sub(out_sl[:64], t1[:64], t2[:64])
                    nc.vector.tensor_add(out_sl[64:], t1[64:], t2[64:])

                for e in range(NE):
                    sl = ts(e, ET)
                    if e == 0:
                        xt_t = xt_e0
                    else:
                        xt_t = BX.tile([128, NDC, ET], f32r, tag="xt")
                        nc.sync.dma_start(out=xt_t, in_=xtr[:, :, sl])
                    # K eighth
                    kp = PS1.tile([128, ET], f32, tag="pp")
                    for c in range(NDC):
                        nc.tensor.matmul(kp, wk_sb[:, c, :], xt_t[:, c, :],
                                         start=(c == 0), stop=(c == NDC - 1))
                    normrope(kp, wkc, wke, sl, KT[:, sl])
                    # V eighth: project transposed then PE-transpose to natural
                    vp = PS1.tile([128, ET], f32, tag="pp")
                    for c in range(NDC):
                        nc.tensor.matmul(vp, wv_sb[:, c, :], xt_t[:, c, :],
                                         start=(c == 0), stop=(c == NDC - 1))
                    vt = TMP.tile([128, ET], f32, tag="vt")
                    nc.scalar.activation(vt, vp, AF.Copy)
                    for i in range(ET // 128):
                        tp = PS1b.tile([128, 128], f32, tag="tp")
                        nc.tensor.transpose(tp, vt[:, ts(i, 128)], ident)
                        nc.scalar.activation(Vn[:, e * (ET // 128) + i, :], tp, AF.Copy)
                    # Q heads
                    for h in range(GQ):
                        qp = PS1.tile([128, ET], f32, tag="pp")
                        for c in range(NDC):
                            nc.tensor.matmul(qp, wq_sb[:, c, ts(h, 128)], xt_t[:, c, :],
                                             start=(c == 0), stop=(c == NDC - 1))
                        normrope(qp, wqc, wqe, sl, QT[:, h, sl])

            with (
                tc.tile_pool(name="C", bufs=1) as C,
                tc.tile_pool(name="CP", bufs=4) as CP,
                tc.tile_pool(name="CT", bufs=3) as CT,
                tc.tile_pool(name="CO", bufs=2) as CO,
            ):
                OT = C.tile([128, GQ, T], f32r, tag="OT")
                wo_sb = C.tile([128, GQ, D], f32r, tag="wo")
                for c in range(GQ):
                    nc.sync.dma_start(out=wo_sb[:, c, :], in_=wor[:, c, :])

                with (
                    tc.tile_pool(name="PS2", bufs=2, space="PSUM") as PS2,
                    tc.tile_pool(name="PS2b", bufs=2, space="PSUM") as PS2b,
                    tc.tile_pool(name="PS3", bufs=2, space="PSUM") as PS3,
                ):
                    for J in range(NJ):
                        nkb = 4 * J + 4
                        for h in range(GQ):
                            lp = PS2b.tile([128, JW], f32, tag="l")
                            op = PS2b.tile([128, JW], f32, tag="ot")
                            for kb in range(nkb):
                                sp = PS2.tile([128, JW], f32, tag="s")
                                nc.tensor.matmul(sp, KT[:, ts(kb, 128)],
                                                 QT[:, h, ts(J, JW)],
                                                 start=True, stop=True)
                                P = CP.tile([128, JW], f32r, tag="p")
                                nc.scalar.activation(P, sp, AF.Exp, scale=SCALE)
                                if kb >= 4 * J:
                                    nc.vector.tensor_mul(P, P, msk_sb[:, kb - 4 * J, :])
                                nc.tensor.matmul(lp, ones_sb, P,
                                                 start=(kb == 0), stop=(kb == nkb - 1))
                                nc.tensor.matmul(op, Vn[:, kb, :], P,
                                                 start=(kb == 0), stop=(kb == nkb - 1))
                            rc2 = CT.tile([128, JW], f32, tag="rc2")
                            nc.vector.reciprocal(rc2, lp)
                            nc.vector.tensor_mul(OT[:, h, ts(J, JW)], op, rc2)
                        # output projection for this J's four q-tiles (overlaps next J)
                        for qt in range(4 * J, 4 * J + 4):
                            ost = CO.tile([128, D], f32, tag="ost")
                            for c in range(D // JW):
                                oup = PS3.tile([128, JW], f32, tag="op")
                                for hc in range(GQ):
                                    nc.tensor.matmul(oup, OT[:, hc, ts(qt, 128)],
                                                     wo_sb[:, hc, ts(c, JW)],
                                                     start=(hc == 0), stop=(hc == GQ - 1))
                                nc.scalar.activation(ost[:, ts(c, JW)], oup, AF.Copy)
                            nc.sync.dma_start(out=outd[qt * 128:(qt + 1) * 128, :], in_=ost)

    nc.finalize()
    return nc


def _host_consts():
    inv = 1.0 / (ROPE_BASE ** (np.arange(0, HD, 2, dtype=np.float64) / HD))
    freqs = np.outer(np.arange(T, dtype=np.float64), inv)
    emb = np.concatenate([freqs, freqs], axis=-1)          # [T, HD]
    cosT = np.ascontiguousarray(np.cos(emb).T.astype(np.float32))  # [HD, T]
    sinT = np.ascontiguousarray(np.sin(emb).T.astype(np.float32))
    msk = np.zeros((4, 128, JW), np.float32)
    for r in range(4):
        k = np.arange(128)[:, None] + 128 * r
        q = np.arange(JW)[None, :]
        msk[r] = (k <= q).astype(np.float32)
    ones = np.ones((128, 128), np.float32)
    return cosT, sinT, msk, ones


def kernel(x, Wq, Wk, Wv, Wo, q_norm_w, k_norm_w):
    from concourse.bass_utils import run_bass_kernel_spmd

    if "nc" not in _cached:
        _cached["nc"] = _build_program()
        _cached["consts"] = _host_consts()
    nc = _cached["nc"]
    cosT, sinT, msk, ones = _cached["consts"]

    x = np.asarray(x, np.float32)
    Wq = np.asarray(Wq, np.float32)
    Wk = np.asarray(Wk, np.float32)
    Wv = np.asarray(Wv, np.float32)
    Wo = np.asarray(Wo, np.float32)
    qwf = np.asarray(q_norm_w, np.float64).reshape(HD, 1)
    kwf = np.asarray(k_norm_w, np.float64).reshape(HD, 1)
    qw = np.ascontiguousarray((1.0 / qwf ** 2).astype(np.float32))
    kw = np.ascontiguousarray((1.0 / kwf ** 2).astype(np.float32))
    qwe = np.ascontiguousarray((EPS / qwf ** 2).astype(np.float32))
    kwe = np.ascontiguousarray((EPS / kwf ** 2).astype(np.float32))

    xTb = [np.ascontiguousarray(x[b].T) for b in range(B)]
    in_maps = []
    for core in range(8):
        b, kv = divmod(core, NKV)
        in_maps.append({
            "xt": xTb[b],
            "wq": np.ascontiguousarray(Wq[:, kv * HQ:(kv + 1) * HQ]),
            "wk": np.ascontiguousarray(Wk[:, kv * HD:(kv + 1) * HD]),
            "wv": np.ascontiguousarray(Wv[:, kv * HD:(kv + 1) * HD]),
            "wo": np.ascontiguousarray(Wo[kv * HQ:(kv + 1) * HQ, :]),
            "cos": cosT, "sin": sinT, "wqc": qw, "wkc": kw, "wqe": qwe, "wke": kwe,
            "msk": msk, "ones": ones, "onesn": ones / HD,
        })
    res = run_bass_kernel_spmd(nc, in_maps, list(range(8)))
    out = np.zeros((B, T, D), np.float64)
    for core in range(8):
        b = core // NKV
        out[b] += res.results[core]["out"].astype(np.float64)
    return out.astype(np.float32)
